# revision 31
# baseline (speedup 1.0000x reference)
"""Trainium2 Bass kernel for the EnhancedMathematicalReasoning MoE-routing module.

Computation (per token t, hidden dim H=2048, E=8 experts, dense routing):
    a1     = gelu(h @ Wd1 + bd1)
    logits = a1 @ Wd2 + bd2
    op_w   = softmax(logits)
    comb   = sum_e op_w[:, e] * (h @ We[e] + be[e])
    out    = (gelu(comb @ Wi1 + bi1) @ Wi2 + bi2) * mask

Sharding: data-parallel over the 8192 tokens -> 1024 tokens per NeuronCore,
weights replicated, no collectives.

Layout strategy (P=128):
  - The host pre-transposes h to [H, T] per core and packs every weight into
    the [P, NB, KT, 256] chunk layout the kernel DMAs, so the device does
    ZERO layout transposes: activations live as [H-on-partitions, T] from
    first GEMM to final store, and the output is written [H, T] and
    transposed back on the host.
  - ALL four GEMM stages run with bf16 operands at N=512 (fp32 PSUM
    accumulation).  bf16 matmuls pace at the 216 ns warm floor because the
    fast-weight-load path engages (fp32r paces at 227 ns/MM, LDW-limited),
    and bf16 halves all weight DMA.
  - Expert outputs are combined op_w-weighted into an fp32 arena (DVE
    mult+add per psum); the LAST expert's accumulate writes the bf16 copy
    (arenaB) that GEMM3 streams, so the 8-way accumulation itself stays
    fp32 with no extra cast pass.
  - op_w[t, e] is broadcast across partitions with a K=8 selector matmul
    (sel8), and the attention mask is broadcast across partitions once with
    a K=1 ones matmul; the GEMM4 eviction is ACT(bias) + DVE(mask mult).
  - Startup: one accumulation group of dummy matmuls warms the PE HAM
    clock-gate while hT and Wd1-chunk-0 transfer concurrently on the two
    hardware DMA queues (Sync + Activation); first real MM at ~12 us warm.
"""

import numpy as np
import ml_dtypes
from contextlib import ExitStack

import concourse.bass as bass
import concourse.tile as tile
from concourse import bacc, mybir
from concourse.bass_utils import run_bass_kernel_spmd
from concourse.masks import make_identity

F32 = mybir.dt.float32
F32R = mybir.dt.float32r
BF16 = mybir.dt.bfloat16
AF = mybir.ActivationFunctionType
ALU = mybir.AluOpType
AX = mybir.AxisListType

P = 128
N_CORES = 8

B, S, H_FULL, E_FULL = 4, 2048, 2048, 8


def build_nc(T, H, E, act=AF.Gelu, include_be=False):
    """Build + compile the single-core program (same program runs SPMD on all
    cores). T: tokens per core. Requires T % 512 == 0, H % 512 == 0."""
    assert T % 512 == 0 and H % 512 == 0 and 2 <= E <= P
    KT = H // P          # contraction chunks of 128
    TT = T // P          # token chunks of 128
    TB = T // 512        # 512-wide moving blocks
    NB = H // 256        # 256-wide weight chunks
    MT = H // P          # output chunks of 128

    nc = bacc.Bacc("TRN2", target_bir_lowering=False, debug=False)

    hT_d = nc.dram_tensor("hT", [T // 512, P, KT, 512], BF16,
                          kind="ExternalInput").ap()
    msk_d = nc.dram_tensor("mask", [1, T], F32R, kind="ExternalInput").ap()
    wd1_d = nc.dram_tensor("wd1", [P, NB, KT, 256], BF16, kind="ExternalInput").ap()
    bd1_d = nc.dram_tensor("bd1", [P, KT], F32, kind="ExternalInput").ap()
    wd2_d = nc.dram_tensor("wd2", [P, KT, E], BF16, kind="ExternalInput").ap()
    bd2_d = nc.dram_tensor("bd2", [1, E], F32, kind="ExternalInput").ap()
    we_d = nc.dram_tensor("we", [E, P, NB, KT, 256], BF16, kind="ExternalInput").ap()
    be_d = nc.dram_tensor("be", [E, H], F32R, kind="ExternalInput").ap()
    wi1_d = nc.dram_tensor("wi1", [P, NB, KT, 256], BF16, kind="ExternalInput").ap()
    bi1_d = nc.dram_tensor("bi1", [P, KT], F32, kind="ExternalInput").ap()
    wi2_d = nc.dram_tensor("wi2", [P, NB, KT, 256], BF16, kind="ExternalInput").ap()
    bi2_d = nc.dram_tensor("bi2", [P, KT], F32, kind="ExternalInput").ap()
    out_d = nc.dram_tensor("out", [P, MT, T], F32, kind="ExternalOutput").ap()

    with tile.TileContext(nc) as tc:
        with ExitStack() as ctx:
            const = ctx.enter_context(tc.tile_pool(name="const", bufs=1))
            bigp = ctx.enter_context(tc.tile_pool(name="bigp", bufs=1))
            web = ctx.enter_context(tc.tile_pool(name="web", bufs=3))
            a1p = ctx.enter_context(tc.tile_pool(name="a1p", bufs=3))
            scr = ctx.enter_context(tc.tile_pool(name="scr", bufs=3))
            opb = ctx.enter_context(tc.tile_pool(name="opb", bufs=3))
            smp = ctx.enter_context(tc.tile_pool(name="smp", bufs=4))
            pp = ctx.enter_context(tc.tile_pool(name="pp", bufs=2, space="PSUM"))

            # ---- constants (engine-generated; no DMA) ----
            ident = const.tile([P, P], F32, name="ident")
            make_identity(nc, ident)
            ones1 = const.tile([1, P], F32, name="ones1")
            nc.vector.memset(ones1, 1.0)
            # sel8[e', e*128+p] = (e' == e): K=8 selector used to broadcast
            # op_w rows across all 128 partitions via a tiny matmul.
            sel8f = const.tile([E, E, P], F32, name="sel8f")
            nc.gpsimd.memset(sel8f, 0.0)
            nc.gpsimd.affine_select(
                out=sel8f, in_=sel8f, compare_op=ALU.not_equal, fill=1.0,
                base=0, pattern=[[-1, E], [0, P]], channel_multiplier=1)
            sel8 = const.tile([E, E * P], F32R, name="sel8")
            nc.scalar.copy(sel8, sel8f.rearrange("e a p -> e (a p)"))
            opwT = const.tile([E, T], F32R, name="opwT")
            opw = const.tile([P, TT, E], F32, name="opw")
            lacc = const.tile([P, TT, E], F32, name="lacc")
            wmup5 = const.tile([P, 512], BF16, name="wmup5")
            nc.vector.memset(wmup5, 0.0)
            wmup = wmup5[:, 0:P]

            # ---- HAM warmup: one accumulation group of dummy matmuls (they
            # pipeline back-to-back with no WAW serialization) keeps the PE
            # activity monitor busy during the ~9us initial DMA so the first
            # real GEMM runs warm.  Borrows the logits psum bank.
            wps = pp.tile([P, 512], F32, tag="lgt", bufs=1, name="wps")
            for i in range(24):
                nc.tensor.matmul(wps, wmup, wmup5, start=(i == 0),
                                 stop=(i == 23))

            # ---- critical-path DMAs first, balanced across the two hardware
            # DMA queues (Sync and Activation, independent bandwidth) so the
            # first GEMM's inputs (Wd1 chunk 0 + the hT tb=0 half) land in
            # ~4.5us of transfer instead of ~6 ----
            hT = bigp.tile([P, KT, T], BF16, tag="A", name="hT")
            w1_0 = web.tile([P, KT, 256], BF16, tag="web", name="wd1m_0")
            nc.sync.dma_start(hT[:, :, 0:384], hT_d[0][:, :, 0:384])
            nc.scalar.dma_start(w1_0, wd1_d[:, 0])
            nc.scalar.dma_start(hT[:, :, 384:512], hT_d[0][:, :, 384:512])
            bd1_t = const.tile([P, KT], F32, name="bd1_t")
            nc.scalar.dma_start(bd1_t, bd1_d)
            wd2_t = const.tile([P, KT, E], BF16, name="wd2_t")
            nc.scalar.dma_start(wd2_t, wd2_d)
            bd2_t = const.tile([1, E], F32, name="bd2_t")
            nc.scalar.dma_start(bd2_t, bd2_d)
            for tb in range(1, TB):
                nc.sync.dma_start(hT[:, :, tb * 512:(tb + 1) * 512], hT_d[tb])

            # ---- mask broadcast across partitions: maskb[p, t] = mask[t] ----
            mask_sb = const.tile([1, T], F32R, name="mask_sb")
            nc.sync.dma_start(mask_sb, msk_d)
            maskb = const.tile([P, T], F32, name="maskb")
            ones1R = const.tile([1, P], F32R, name="ones1R")
            nc.scalar.copy(ones1R, ones1)
            for tb in range(TB):
                mbps = pp.tile([P, 512], F32, tag="tr", bufs=2, name="mbps")
                nc.tensor.matmul(mbps, ones1R,
                                 mask_sb[:, tb * 512:(tb + 1) * 512],
                                 start=True, stop=True)
                nc.scalar.copy(maskb[:, tb * 512:(tb + 1) * 512], mbps)

            # ---- remaining small constants ----
            bi1_t = const.tile([P, KT], F32, name="bi1_t")
            nc.sync.dma_start(bi1_t, bi1_d)
            bi2_t = const.tile([P, KT], F32, name="bi2_t")
            nc.sync.dma_start(bi2_t, bi2_d)
            if include_be:
                be_t = const.tile([E, H], F32R, name="be_t")
                nc.sync.dma_start(be_t, be_d)

            # ---- stage B: a1 = act(Wd1.T @ hT + bd1) + fused logits GEMM ----
            # tb-outer within each weight chunk so the first 32 MMs only need
            # hT's tb=0 half (the tb=1 DMA is still in flight at kernel start)
            for mg in range(NB):
                if mg == 0:
                    w1 = w1_0
                else:
                    w1 = web.tile([P, KT, 256], BF16, tag="web", name=f"wd1m_{mg}")
                    nc.sync.dma_start(w1, wd1_d[:, mg])
                a1s = {}
                for mi in range(2):
                    m = 2 * mg + mi
                    a1s[mi] = a1p.tile([P, T], BF16, tag="a1", bufs=3,
                                       name=f"a1_{m}")
                for tb in range(TB):
                    for mi in range(2):
                        m = 2 * mg + mi
                        ps = pp.tile([P, 512], F32, tag="mm", bufs=5, name="ps_g1")
                        for k in range(KT):
                            nc.tensor.matmul(ps, w1[:, k, mi * P:(mi + 1) * P],
                                             hT[:, k, tb * 512:(tb + 1) * 512],
                                             start=(k == 0), stop=(k == KT - 1))
                        nc.scalar.activation(a1s[mi][:, tb * 512:(tb + 1) * 512],
                                             ps, act, bias=bd1_t[:, m:m + 1])
                for mi in range(2):
                    m = 2 * mg + mi
                    a1 = a1s[mi]
                    for tt in range(TT):
                        lg = pp.tile([P, E], F32, tag="lgt", bufs=1, name="lg")
                        nc.tensor.matmul(lg, a1[:, tt * P:(tt + 1) * P],
                                         wd2_t[:, m, :], start=True, stop=(m > 0))
                        if m == 0:
                            nc.tensor.matmul(lg, ones1, bd2_t, start=False,
                                             stop=True)
                            nc.vector.tensor_copy(lacc[:, tt, :], lg)
                        else:
                            nc.vector.tensor_add(lacc[:, tt, :], lacc[:, tt, :], lg)

            # ---- softmax over E ----
            for tt in range(TT):
                nmax = smp.tile([P, 1], F32, tag="sm1", bufs=6, name="nmax")
                nc.vector.reduce_max(nmax, lacc[:, tt, :], AX.X, negate=True)
                et = smp.tile([P, E], F32, tag="sme", bufs=2, name="et")
                nc.scalar.activation(et, lacc[:, tt, :], AF.Exp, bias=nmax,
                                     scale=1.0)
                ssum = smp.tile([P, 1], F32, tag="sm1", bufs=6, name="ssum")
                nc.vector.reduce_sum(ssum, et, AX.X)
                rin = smp.tile([P, 1], F32, tag="sm1", bufs=6, name="rin")
                nc.vector.reciprocal(rin, ssum)
                nc.vector.tensor_scalar_mul(opw[:, tt, :], et, rin)
            # Hoisted: expert-0 chunk-0 psum fills (m=0,1 x tb) depend only on
            # hT and We[0], so they keep the PE busy while the serial softmax
            # -> op_w-transpose -> broadcast chain resolves on DVE/ACT.
            wet00 = web.tile([P, KT, 256], BF16, tag="web", name="we_0_0")
            nc.sync.dma_start(wet00, we_d[0, :, 0])
            hoist_ps = {}
            for mi in range(2):
                for tb in range(TB):
                    ps = pp.tile([P, 512], F32, tag="mm", bufs=5, name="eps_h")
                    for k in range(KT):
                        nc.tensor.matmul(ps, wet00[:, k, mi * P:(mi + 1) * P],
                                         hT[:, k, tb * 512:(tb + 1) * 512],
                                         start=(k == 0), stop=(k == KT - 1))
                    hoist_ps[(mi, tb)] = ps
            for tt in range(TT):
                trp = pp.tile([P, P], F32, tag="tr", bufs=2, name="trp_ow")
                nc.tensor.matmul(trp[:E, :], opw[:, tt, :], ident,
                                 is_transpose=True, start=True, stop=True)
                nc.scalar.copy(opwT[:, tt * P:(tt + 1) * P], trp[:E, :])

            # ---- stage C: expert GEMMs, [H_out, T] psums.
            # op_w[t, e] is broadcast across partitions as ob = sel8[:, e].T @
            # opwT (a K=8 matmul), and the weighted combine accumulates into
            # the fp32 arena combT [H, T]:
            #     arena[m, t] (+)= ob[t] * psum[m, t]
            arena = bigp.tile([P, KT, TT, P], F32, tag="B", name="arena")
            arenaR = arena.bitcast(F32R)
            # bf16 copy of the finished comb, written directly by the LAST
            # expert's accumulate (no separate cast pass); GEMM3's moving
            # operand, laid out like hT: [P, KT, T]
            arenaB = bigp.tile([P, KT, T], BF16, tag="C", name="arenaB")

            if include_be:
                # init combT with the op_w-weighted bias term:
                #   arena[m*128+p, t] = sum_e op_w[t, e] * be[e, m*128+p]
                for m in range(MT):
                    for tb in range(TB):
                        bps = pp.tile([P, 512], F32, tag="mm", bufs=5, name="bps")
                        nc.tensor.matmul(bps, be_t[:, m * P:(m + 1) * P],
                                         opwT[:, tb * 512:(tb + 1) * 512],
                                         start=True, stop=True)
                        nc.scalar.copy(
                            arenaR[:, m, tb * 4:(tb + 1) * 4, :],
                            bps.rearrange("p (n c) -> p n c", c=P))

            obs = {}
            for e in range(E):
                for tb in range(TB):
                    bps = pp.tile([P, 512], F32, tag="tr", bufs=2, name="bps")
                    nc.tensor.matmul(bps, sel8[:, e * P:(e + 1) * P],
                                     opwT[:, tb * 512:(tb + 1) * 512],
                                     start=True, stop=True)
                    ob = opb.tile([P, 512], F32, tag="ob", bufs=3,
                                  name=f"ob_{e}_{tb}")
                    nc.scalar.copy(ob, bps)
                    obs[tb] = ob
                for mg in range(NB):
                    if e == 0 and mg == 0:
                        wet = wet00
                    else:
                        wet = web.tile([P, KT, 256], BF16, tag="web",
                                       name=f"we_{e}_{mg}")
                        nc.sync.dma_start(wet, we_d[e, :, mg])
                    for mi in range(2):
                        m = 2 * mg + mi
                        for tb in range(TB):
                            if e == 0 and mg == 0:
                                ps = hoist_ps[(mi, tb)]
                            else:
                                ps = pp.tile([P, 512], F32, tag="mm", bufs=5,
                                             name="eps")
                                for k in range(KT):
                                    nc.tensor.matmul(
                                        ps, wet[:, k, mi * P:(mi + 1) * P],
                                        hT[:, k, tb * 512:(tb + 1) * 512],
                                        start=(k == 0), stop=(k == KT - 1))
                            wsl = arenaR[:, m, tb * 4:(tb + 1) * 4, :]
                            rsl = arena[:, m, tb * 4:(tb + 1) * 4, :]
                            ob3 = obs[tb].rearrange("p (n c) -> p n c", c=P)
                            ps3 = ps.rearrange("p (n c) -> p n c", c=P)
                            if e == 0 and not include_be:
                                nc.vector.tensor_tensor(wsl, ps3, ob3,
                                                        op=ALU.mult)
                            else:
                                tmp = scr.tile([P, 512], F32, tag="s",
                                               bufs=3, name="tmp")
                                tmp3 = tmp.rearrange("p (n c) -> p n c", c=P)
                                nc.vector.tensor_tensor(tmp3, ps3, ob3,
                                                        op=ALU.mult)
                                if e == E - 1:
                                    wb3 = arenaB[
                                        :, m, tb * 512:(tb + 1) * 512
                                    ].rearrange("p (n c) -> p n c", c=P)
                                    nc.vector.tensor_tensor(wb3, rsl, tmp3,
                                                            op=ALU.add)
                                else:
                                    nc.vector.tensor_tensor(wsl, rsl, tmp3,
                                                            op=ALU.add)
                    # prefetch the first Wi1 chunk near the end of the last
                    # expert so stage E doesn't stall on DMA
                    if e == E - 1 and mg == NB - 2:
                        w3_0 = web.tile([P, KT, 256], BF16, tag="web",
                                        name="wi1m_0")
                        nc.sync.dma_start(w3_0, wi1_d[:, 0])

            # ---- stage E: a2T = act(Wi1.T @ arenaB + bi1), bf16 ----
            a2T = bigp.tile([P, KT, T], BF16, tag="A", name="a2T")
            for mg in range(NB):
                if mg == 0:
                    w3 = w3_0
                else:
                    w3 = web.tile([P, KT, 256], BF16, tag="web",
                                  name=f"wi1m_{mg}")
                    nc.sync.dma_start(w3, wi1_d[:, mg])
                for mi in range(2):
                    m = 2 * mg + mi
                    for tb in range(TB):
                        ps = pp.tile([P, 512], F32, tag="mm", bufs=5, name="ps_g3")
                        for k in range(KT):
                            nc.tensor.matmul(ps, w3[:, k, mi * P:(mi + 1) * P],
                                             arenaB[:, k, tb * 512:(tb + 1) * 512],
                                             start=(k == 0), stop=(k == KT - 1))
                        nc.scalar.activation(a2T[:, m, tb * 512:(tb + 1) * 512],
                                             ps, act, bias=bi1_t[:, m:m + 1])
                # prefetch the first Wi2 chunk before stage F
                if mg == NB - 2:
                    w4_0 = web.tile([P, KT, 256], BF16, tag="web", name="wi2m_0")
                    nc.sync.dma_start(w4_0, wi2_d[:, 0])

            # ---- stage F: out = (Wi2.T @ a2T + bi2) * maskb, stored [H, T] ----
            for mg in range(NB):
                if mg == 0:
                    w4 = w4_0
                else:
                    w4 = web.tile([P, KT, 256], BF16, tag="web", name=f"wi2m_{mg}")
                    nc.sync.dma_start(w4, wi2_d[:, mg])
                for mi in range(2):
                    m = 2 * mg + mi
                    for tb in range(TB):
                        ps = pp.tile([P, 512], F32, tag="mm", bufs=5, name="ps_g4")
                        for k in range(KT):
                            nc.tensor.matmul(ps, w4[:, k, mi * P:(mi + 1) * P],
                                             a2T[:, k, tb * 512:(tb + 1) * 512],
                                             start=(k == 0), stop=(k == KT - 1))
                        og = scr.tile([P, 512], F32, tag="og", bufs=3, name="og")
                        nc.scalar.activation(og, ps, AF.Identity,
                                             bias=bi2_t[:, m:m + 1])
                        og2 = scr.tile([P, 512], F32, tag="og2", bufs=3,
                                       name="og2")
                        nc.vector.tensor_tensor(
                            og2, og, maskb[:, tb * 512:(tb + 1) * 512],
                            op=ALU.mult)
                        nc.sync.dma_start(
                            out_d[:, m, tb * 512:(tb + 1) * 512], og2)

    nc.compile()
    return nc


_CACHED = {}


def _get_nc(T, H, E, include_be):
    key = (T, H, E, include_be)
    if key not in _CACHED:
        _CACHED[key] = build_nc(T, H, E, act=AF.Gelu, include_be=include_be)
    return _CACHED[key]


def _pack_w(W, dtype):
    """[H_in, N_out] -> [P, NB, KT, 256] chunk layout (contiguous per DMA)."""
    H_in, N_out = W.shape
    KT, NB = H_in // P, N_out // 256
    Wp = W.reshape(KT, P, NB, 256).transpose(1, 2, 0, 3)
    return np.ascontiguousarray(Wp.astype(dtype))


def kernel(hidden_states, attention_mask, Wd1, bd1, Wd2, bd2, We, be, Wi1, bi1,
           Wi2, bi2, _trace=False):
    f32 = lambda x: np.ascontiguousarray(np.asarray(x, dtype=np.float32))
    bf16 = ml_dtypes.bfloat16
    h = f32(hidden_states)
    mask = f32(attention_mask)
    Wd1, bd1, Wd2, bd2 = f32(Wd1), f32(bd1), f32(Wd2), f32(bd2)
    We, be, Wi1, bi1, Wi2, bi2 = f32(We), f32(be), f32(Wi1), f32(bi1), f32(Wi2), f32(bi2)

    Bv, Sv, Hv = h.shape
    Ev = Wd2.shape[1]
    TOK = Bv * Sv
    T = TOK // N_CORES
    KT = Hv // P
    include_be = bool(np.any(be))

    nc = _get_nc(T, Hv, Ev, include_be)

    hf = h.reshape(TOK, Hv)
    mf = mask.reshape(TOK)

    # Wd2 is [H, E]: pack to [P, KT, E]
    wd2_p = np.ascontiguousarray(
        Wd2.reshape(KT, P, Ev).transpose(1, 0, 2).astype(bf16))
    bias_p = lambda b: np.ascontiguousarray(b.reshape(KT, P).T)
    weights = dict(
        wd1=_pack_w(Wd1, bf16), bd1=bias_p(bd1),
        wd2=wd2_p, bd2=bd2.reshape(1, Ev),
        we=np.ascontiguousarray(
            np.stack([_pack_w(We[e], bf16) for e in range(Ev)])),
        be=be,
        wi1=_pack_w(Wi1, bf16), bi1=bias_p(bi1),
        wi2=_pack_w(Wi2, bf16), bi2=bias_p(bi2),
    )
    in_maps = []
    for c in range(N_CORES):
        m = dict(weights)
        hc = hf[c * T:(c + 1) * T]                        # [T, H]
        # [TB, P, KT, 512]: per-partition-contiguous 512-token blocks
        hTc = hc.T.reshape(KT, P, T // 512, 512).transpose(2, 1, 0, 3)
        m["hT"] = np.ascontiguousarray(hTc.astype(bf16))
        m["mask"] = np.ascontiguousarray(mf[c * T:(c + 1) * T].reshape(1, T))
        in_maps.append(m)

    # The first execution of a freshly-loaded NEFF occasionally trips a
    # transient NRT_EXEC_UNIT_UNRECOVERABLE on the axon worker; a retry after a
    # short pause has always succeeded, so tolerate a couple of those.
    last_exc = None
    for attempt in range(3):
        try:
            res = run_bass_kernel_spmd(nc, in_maps,
                                       core_ids=list(range(N_CORES)),
                                       trace=_trace)
            break
        except Exception as e:  # noqa: BLE001 - jax.errors.JaxRuntimeError
            last_exc = e
            if "UNAVAILABLE" not in str(e) and "unrecoverable" not in str(e):
                raise
            import time as _time
            _time.sleep(5 * (attempt + 1))
    else:
        raise last_exc
    out = np.empty((TOK, Hv), dtype=np.float32)
    for c in range(N_CORES):
        oc = res.results[c]["out"]                        # [P, MT, T]
        out[c * T:(c + 1) * T] = oc.transpose(1, 0, 2).reshape(Hv, T).T
    out = out.reshape(Bv, Sv, Hv)
    if _trace:
        kernel._last_results = res
    return out


# revision 32
# speedup vs baseline: 1.0060x; 1.0060x over previous
"""Trainium2 Bass kernel for the EnhancedMathematicalReasoning MoE-routing module.

Computation (per token t, hidden dim H=2048, E=8 experts, dense routing):
    a1     = gelu(h @ Wd1 + bd1)
    logits = a1 @ Wd2 + bd2
    op_w   = softmax(logits)
    comb   = sum_e op_w[:, e] * (h @ We[e] + be[e])
    out    = (gelu(comb @ Wi1 + bi1) @ Wi2 + bi2) * mask

Sharding: data-parallel over the 8192 tokens -> 1024 tokens per NeuronCore,
weights replicated, no collectives.

v2 layout strategy (P=128):
  - The host pre-transposes h to [H, T] per core and packs every weight into
    the [P, NB, KT, 256] chunk layout the kernel DMAs, so the device does
    ZERO layout transposes: activations live as [H-on-partitions, T] from
    first GEMM to final store, and the output is written [H, T] and
    transposed back on the host.
  - GEMM1 (Wd1), the 8 expert GEMMs (We) and GEMM4 (Wi2) run in bf16
    (fp32 PSUM accumulation).  bf16 enables the PE fast-weight-load path,
    which is what paces an fp32r matmul stream (fp32r: 227 ns/MM with the
    187 ns self-loaded LDWEIGHTS trailing each MM; bf16 LDW is ~2x faster),
    and it halves all weight DMA.
  - GEMM3 (Wi1) stays fp32r and streams the fp32 expert-combine arena
    directly, so the op_w-weighted accumulation over experts keeps full
    fp32 precision end-to-end.
  - op_w[t, e] is broadcast across partitions with a K=8 selector matmul
    (sel8), and the attention mask is broadcast across partitions once with
    a K=1 ones matmul; the GEMM4 eviction is ACT(bias) + DVE(mask mult).
  - A short burst of dummy matmuls at kernel start keeps the PE HAM
    activity monitor busy during the initial h/weight DMA so the first
    real GEMM runs at 2.4 GHz instead of the cold 1.2 GHz.
"""

import numpy as np
import ml_dtypes
from contextlib import ExitStack

import concourse.bass as bass
import concourse.tile as tile
from concourse import bacc, mybir
from concourse.bass_utils import run_bass_kernel_spmd
from concourse.masks import make_identity

F32 = mybir.dt.float32
F32R = mybir.dt.float32r
BF16 = mybir.dt.bfloat16
AF = mybir.ActivationFunctionType
ALU = mybir.AluOpType
AX = mybir.AxisListType

P = 128
N_CORES = 8

B, S, H_FULL, E_FULL = 4, 2048, 2048, 8


def build_nc(T, H, E, act=AF.Gelu, include_be=False):
    """Build + compile the single-core program (same program runs SPMD on all
    cores). T: tokens per core. Requires T % 512 == 0, H % 512 == 0."""
    assert T % 512 == 0 and H % 512 == 0 and 2 <= E <= P
    KT = H // P          # contraction chunks of 128
    TT = T // P          # token chunks of 128
    TB = T // 512        # 512-wide moving blocks
    NB = H // 256        # 256-wide weight chunks
    MT = H // P          # output chunks of 128

    nc = bacc.Bacc("TRN2", target_bir_lowering=False, debug=False)

    hT_d = nc.dram_tensor("hT", [T // 512, P, KT, 512], BF16,
                          kind="ExternalInput").ap()
    msk_d = nc.dram_tensor("mask", [1, T], F32R, kind="ExternalInput").ap()
    wd1_d = nc.dram_tensor("wd1", [P, NB, KT, 256], BF16, kind="ExternalInput").ap()
    bd1_d = nc.dram_tensor("bd1", [P, KT], F32, kind="ExternalInput").ap()
    wd2_d = nc.dram_tensor("wd2", [P, KT, E], BF16, kind="ExternalInput").ap()
    bd2_d = nc.dram_tensor("bd2", [1, E], F32, kind="ExternalInput").ap()
    we_d = nc.dram_tensor("we", [E, P, NB, KT, 256], BF16, kind="ExternalInput").ap()
    be_d = nc.dram_tensor("be", [E, H], F32R, kind="ExternalInput").ap()
    wi1_d = nc.dram_tensor("wi1", [P, NB, KT, 256], BF16, kind="ExternalInput").ap()
    bi1_d = nc.dram_tensor("bi1", [P, KT], F32, kind="ExternalInput").ap()
    wi2_d = nc.dram_tensor("wi2", [P, NB, KT, 256], BF16, kind="ExternalInput").ap()
    bi2_d = nc.dram_tensor("bi2", [P, KT], F32, kind="ExternalInput").ap()
    out_d = nc.dram_tensor("out", [P, MT, T], F32, kind="ExternalOutput").ap()

    with tile.TileContext(nc) as tc:
        with ExitStack() as ctx:
            const = ctx.enter_context(tc.tile_pool(name="const", bufs=1))
            bigp = ctx.enter_context(tc.tile_pool(name="bigp", bufs=1))
            web = ctx.enter_context(tc.tile_pool(name="web", bufs=3))
            a1p = ctx.enter_context(tc.tile_pool(name="a1p", bufs=3))
            scr = ctx.enter_context(tc.tile_pool(name="scr", bufs=3))
            opb = ctx.enter_context(tc.tile_pool(name="opb", bufs=3))
            smp = ctx.enter_context(tc.tile_pool(name="smp", bufs=4))
            pp = ctx.enter_context(tc.tile_pool(name="pp", bufs=2, space="PSUM"))

            # ---- constants (engine-generated; no DMA) ----
            ident = const.tile([P, P], F32, name="ident")
            make_identity(nc, ident)
            ones1 = const.tile([1, P], F32, name="ones1")
            nc.vector.memset(ones1, 1.0)
            # sel8[e', e*128+p] = (e' == e): K=8 selector used to broadcast
            # op_w rows across all 128 partitions via a tiny matmul.
            sel8f = const.tile([E, E, P], F32, name="sel8f")
            nc.gpsimd.memset(sel8f, 0.0)
            nc.gpsimd.affine_select(
                out=sel8f, in_=sel8f, compare_op=ALU.not_equal, fill=1.0,
                base=0, pattern=[[-1, E], [0, P]], channel_multiplier=1)
            sel8 = const.tile([E, E * P], F32R, name="sel8")
            nc.scalar.copy(sel8, sel8f.rearrange("e a p -> e (a p)"))
            opwT = const.tile([E, T], F32R, name="opwT")
            opw = const.tile([P, TT, E], F32, name="opw")
            lacc = const.tile([P, TT, E], F32, name="lacc")
            wmup5 = const.tile([P, 512], BF16, name="wmup5")
            nc.vector.memset(wmup5, 0.0)
            wmup = wmup5[:, 0:P]

            # ---- HAM warmup: one accumulation group of dummy matmuls (they
            # pipeline back-to-back with no WAW serialization) keeps the PE
            # activity monitor busy during the ~9us initial DMA so the first
            # real GEMM runs warm.  Borrows the logits psum bank.
            wps = pp.tile([P, 512], F32, tag="lgt", bufs=1, name="wps")
            for i in range(32):
                nc.tensor.matmul(wps, wmup, wmup5, start=(i == 0),
                                 stop=(i == 31))

            # ---- critical-path DMAs first, split across the two hardware
            # DMA queues (Sync and Activation) so hT and Wd1 chunk 0
            # transfer concurrently ----
            hT = bigp.tile([P, KT, T], BF16, tag="A", name="hT")
            w1_0 = web.tile([P, KT, 256], BF16, tag="web", name="wd1m_0")
            nc.sync.dma_start(hT[:, :, 0:512], hT_d[0])
            nc.scalar.dma_start(w1_0, wd1_d[:, 0])
            bd1_t = const.tile([P, KT], F32, name="bd1_t")
            nc.scalar.dma_start(bd1_t, bd1_d)
            wd2_t = const.tile([P, KT, E], BF16, name="wd2_t")
            nc.scalar.dma_start(wd2_t, wd2_d)
            bd2_t = const.tile([1, E], F32, name="bd2_t")
            nc.scalar.dma_start(bd2_t, bd2_d)
            for tb in range(1, TB):
                nc.sync.dma_start(hT[:, :, tb * 512:(tb + 1) * 512], hT_d[tb])

            # ---- mask broadcast across partitions: maskb[p, t] = mask[t] ----
            mask_sb = const.tile([1, T], F32R, name="mask_sb")
            nc.sync.dma_start(mask_sb, msk_d)
            maskb = const.tile([P, T], F32, name="maskb")
            ones1R = const.tile([1, P], F32R, name="ones1R")
            nc.scalar.copy(ones1R, ones1)
            for tb in range(TB):
                mbps = pp.tile([P, 512], F32, tag="tr", bufs=2, name="mbps")
                nc.tensor.matmul(mbps, ones1R,
                                 mask_sb[:, tb * 512:(tb + 1) * 512],
                                 start=True, stop=True)
                nc.scalar.copy(maskb[:, tb * 512:(tb + 1) * 512], mbps)

            # ---- remaining small constants ----
            bi1_t = const.tile([P, KT], F32, name="bi1_t")
            nc.sync.dma_start(bi1_t, bi1_d)
            bi2_t = const.tile([P, KT], F32, name="bi2_t")
            nc.sync.dma_start(bi2_t, bi2_d)
            if include_be:
                be_t = const.tile([E, H], F32R, name="be_t")
                nc.sync.dma_start(be_t, be_d)

            # ---- stage B: a1 = act(Wd1.T @ hT + bd1) + fused logits GEMM ----
            # tb-outer within each weight chunk so the first 32 MMs only need
            # hT's tb=0 half (the tb=1 DMA is still in flight at kernel start)
            for mg in range(NB):
                if mg == 0:
                    w1 = w1_0
                else:
                    w1 = web.tile([P, KT, 256], BF16, tag="web", name=f"wd1m_{mg}")
                    nc.sync.dma_start(w1, wd1_d[:, mg])
                a1s = {}
                for mi in range(2):
                    m = 2 * mg + mi
                    a1s[mi] = a1p.tile([P, T], BF16, tag="a1", bufs=3,
                                       name=f"a1_{m}")
                for tb in range(TB):
                    for mi in range(2):
                        m = 2 * mg + mi
                        ps = pp.tile([P, 512], F32, tag="mm", bufs=5, name="ps_g1")
                        for k in range(KT):
                            nc.tensor.matmul(ps, w1[:, k, mi * P:(mi + 1) * P],
                                             hT[:, k, tb * 512:(tb + 1) * 512],
                                             start=(k == 0), stop=(k == KT - 1))
                        nc.scalar.activation(a1s[mi][:, tb * 512:(tb + 1) * 512],
                                             ps, act, bias=bd1_t[:, m:m + 1])
                for mi in range(2):
                    m = 2 * mg + mi
                    a1 = a1s[mi]
                    for tt in range(TT):
                        lg = pp.tile([P, E], F32, tag="lgt", bufs=1, name="lg")
                        nc.tensor.matmul(lg, a1[:, tt * P:(tt + 1) * P],
                                         wd2_t[:, m, :], start=True, stop=(m > 0))
                        if m == 0:
                            nc.tensor.matmul(lg, ones1, bd2_t, start=False,
                                             stop=True)
                            nc.vector.tensor_copy(lacc[:, tt, :], lg)
                        else:
                            nc.vector.tensor_add(lacc[:, tt, :], lacc[:, tt, :], lg)

            # ---- softmax over E ----
            for tt in range(TT):
                nmax = smp.tile([P, 1], F32, tag="sm1", bufs=6, name="nmax")
                nc.vector.reduce_max(nmax, lacc[:, tt, :], AX.X, negate=True)
                et = smp.tile([P, E], F32, tag="sme", bufs=2, name="et")
                nc.scalar.activation(et, lacc[:, tt, :], AF.Exp, bias=nmax,
                                     scale=1.0)
                ssum = smp.tile([P, 1], F32, tag="sm1", bufs=6, name="ssum")
                nc.vector.reduce_sum(ssum, et, AX.X)
                rin = smp.tile([P, 1], F32, tag="sm1", bufs=6, name="rin")
                nc.vector.reciprocal(rin, ssum)
                nc.vector.tensor_scalar_mul(opw[:, tt, :], et, rin)
            # Hoisted: expert-0 chunk-0 psum fills (m=0,1 x tb) depend only on
            # hT and We[0], so they keep the PE busy while the serial softmax
            # -> op_w-transpose -> broadcast chain resolves on DVE/ACT.
            wet00 = web.tile([P, KT, 256], BF16, tag="web", name="we_0_0")
            nc.sync.dma_start(wet00, we_d[0, :, 0])
            hoist_ps = {}
            for mi in range(2):
                for tb in range(TB):
                    ps = pp.tile([P, 512], F32, tag="mm", bufs=5, name="eps_h")
                    for k in range(KT):
                        nc.tensor.matmul(ps, wet00[:, k, mi * P:(mi + 1) * P],
                                         hT[:, k, tb * 512:(tb + 1) * 512],
                                         start=(k == 0), stop=(k == KT - 1))
                    hoist_ps[(mi, tb)] = ps
            for tt in range(TT):
                trp = pp.tile([P, P], F32, tag="tr", bufs=2, name="trp_ow")
                nc.tensor.matmul(trp[:E, :], opw[:, tt, :], ident,
                                 is_transpose=True, start=True, stop=True)
                nc.scalar.copy(opwT[:, tt * P:(tt + 1) * P], trp[:E, :])

            # ---- stage C: expert GEMMs, [H_out, T] psums.
            # op_w[t, e] is broadcast across partitions as ob = sel8[:, e].T @
            # opwT (a K=8 matmul), and the weighted combine accumulates into
            # the fp32 arena combT [H, T]:
            #     arena[m, t] (+)= ob[t] * psum[m, t]
            arena = bigp.tile([P, KT, TT, P], F32, tag="B", name="arena")
            arenaR = arena.bitcast(F32R)
            # bf16 copy of the finished comb, written directly by the LAST
            # expert's accumulate (no separate cast pass); GEMM3's moving
            # operand, laid out like hT: [P, KT, T]
            arenaB = bigp.tile([P, KT, T], BF16, tag="C", name="arenaB")

            if include_be:
                # init combT with the op_w-weighted bias term:
                #   arena[m*128+p, t] = sum_e op_w[t, e] * be[e, m*128+p]
                for m in range(MT):
                    for tb in range(TB):
                        bps = pp.tile([P, 512], F32, tag="mm", bufs=5, name="bps")
                        nc.tensor.matmul(bps, be_t[:, m * P:(m + 1) * P],
                                         opwT[:, tb * 512:(tb + 1) * 512],
                                         start=True, stop=True)
                        nc.scalar.copy(
                            arenaR[:, m, tb * 4:(tb + 1) * 4, :],
                            bps.rearrange("p (n c) -> p n c", c=P))

            obs = {}
            for e in range(E):
                for tb in range(TB):
                    bps = pp.tile([P, 512], F32, tag="tr", bufs=2, name="bps")
                    nc.tensor.matmul(bps, sel8[:, e * P:(e + 1) * P],
                                     opwT[:, tb * 512:(tb + 1) * 512],
                                     start=True, stop=True)
                    ob = opb.tile([P, 512], F32, tag="ob", bufs=3,
                                  name=f"ob_{e}_{tb}")
                    nc.scalar.copy(ob, bps)
                    obs[tb] = ob
                for mg in range(NB):
                    if e == 0 and mg == 0:
                        wet = wet00
                    else:
                        wet = web.tile([P, KT, 256], BF16, tag="web",
                                       name=f"we_{e}_{mg}")
                        nc.sync.dma_start(wet, we_d[e, :, mg])
                    for mi in range(2):
                        m = 2 * mg + mi
                        for tb in range(TB):
                            if e == 0 and mg == 0:
                                ps = hoist_ps[(mi, tb)]
                            else:
                                ps = pp.tile([P, 512], F32, tag="mm", bufs=5,
                                             name="eps")
                                for k in range(KT):
                                    nc.tensor.matmul(
                                        ps, wet[:, k, mi * P:(mi + 1) * P],
                                        hT[:, k, tb * 512:(tb + 1) * 512],
                                        start=(k == 0), stop=(k == KT - 1))
                            wsl = arenaR[:, m, tb * 4:(tb + 1) * 4, :]
                            rsl = arena[:, m, tb * 4:(tb + 1) * 4, :]
                            ob3 = obs[tb].rearrange("p (n c) -> p n c", c=P)
                            ps3 = ps.rearrange("p (n c) -> p n c", c=P)
                            if e == 0 and not include_be:
                                nc.vector.tensor_tensor(wsl, ps3, ob3,
                                                        op=ALU.mult)
                            else:
                                tmp = scr.tile([P, 512], F32, tag="s",
                                               bufs=3, name="tmp")
                                tmp3 = tmp.rearrange("p (n c) -> p n c", c=P)
                                nc.vector.tensor_tensor(tmp3, ps3, ob3,
                                                        op=ALU.mult)
                                if e == E - 1:
                                    wb3 = arenaB[
                                        :, m, tb * 512:(tb + 1) * 512
                                    ].rearrange("p (n c) -> p n c", c=P)
                                    nc.vector.tensor_tensor(wb3, rsl, tmp3,
                                                            op=ALU.add)
                                else:
                                    nc.vector.tensor_tensor(wsl, rsl, tmp3,
                                                            op=ALU.add)
                    # prefetch the first Wi1 chunk near the end of the last
                    # expert so stage E doesn't stall on DMA
                    if e == E - 1 and mg == NB - 2:
                        w3_0 = web.tile([P, KT, 256], BF16, tag="web",
                                        name="wi1m_0")
                        nc.sync.dma_start(w3_0, wi1_d[:, 0])

            # ---- stage E: a2T = act(Wi1.T @ arenaB + bi1), bf16 ----
            a2T = bigp.tile([P, KT, T], BF16, tag="A", name="a2T")
            for mg in range(NB):
                if mg == 0:
                    w3 = w3_0
                else:
                    w3 = web.tile([P, KT, 256], BF16, tag="web",
                                  name=f"wi1m_{mg}")
                    nc.sync.dma_start(w3, wi1_d[:, mg])
                for mi in range(2):
                    m = 2 * mg + mi
                    for tb in range(TB):
                        ps = pp.tile([P, 512], F32, tag="mm", bufs=5, name="ps_g3")
                        for k in range(KT):
                            nc.tensor.matmul(ps, w3[:, k, mi * P:(mi + 1) * P],
                                             arenaB[:, k, tb * 512:(tb + 1) * 512],
                                             start=(k == 0), stop=(k == KT - 1))
                        nc.scalar.activation(a2T[:, m, tb * 512:(tb + 1) * 512],
                                             ps, act, bias=bi1_t[:, m:m + 1])
                # prefetch the first Wi2 chunk before stage F
                if mg == NB - 2:
                    w4_0 = web.tile([P, KT, 256], BF16, tag="web", name="wi2m_0")
                    nc.sync.dma_start(w4_0, wi2_d[:, 0])

            # ---- stage F: out = (Wi2.T @ a2T + bi2) * maskb, stored [H, T] ----
            for mg in range(NB):
                if mg == 0:
                    w4 = w4_0
                else:
                    w4 = web.tile([P, KT, 256], BF16, tag="web", name=f"wi2m_{mg}")
                    nc.sync.dma_start(w4, wi2_d[:, mg])
                for mi in range(2):
                    m = 2 * mg + mi
                    for tb in range(TB):
                        ps = pp.tile([P, 512], F32, tag="mm", bufs=5, name="ps_g4")
                        for k in range(KT):
                            nc.tensor.matmul(ps, w4[:, k, mi * P:(mi + 1) * P],
                                             a2T[:, k, tb * 512:(tb + 1) * 512],
                                             start=(k == 0), stop=(k == KT - 1))
                        og = scr.tile([P, 512], F32, tag="og", bufs=3, name="og")
                        nc.scalar.activation(og, ps, AF.Identity,
                                             bias=bi2_t[:, m:m + 1])
                        og2 = scr.tile([P, 512], F32, tag="og2", bufs=3,
                                       name="og2")
                        nc.vector.tensor_tensor(
                            og2, og, maskb[:, tb * 512:(tb + 1) * 512],
                            op=ALU.mult)
                        nc.sync.dma_start(
                            out_d[:, m, tb * 512:(tb + 1) * 512], og2)

    nc.compile()
    return nc


_CACHED = {}


def _get_nc(T, H, E, include_be):
    key = (T, H, E, include_be)
    if key not in _CACHED:
        _CACHED[key] = build_nc(T, H, E, act=AF.Gelu, include_be=include_be)
    return _CACHED[key]


def _pack_w(W, dtype):
    """[H_in, N_out] -> [P, NB, KT, 256] chunk layout (contiguous per DMA)."""
    H_in, N_out = W.shape
    KT, NB = H_in // P, N_out // 256
    Wp = W.reshape(KT, P, NB, 256).transpose(1, 2, 0, 3)
    return np.ascontiguousarray(Wp.astype(dtype))


def kernel(hidden_states, attention_mask, Wd1, bd1, Wd2, bd2, We, be, Wi1, bi1,
           Wi2, bi2, _trace=False):
    f32 = lambda x: np.ascontiguousarray(np.asarray(x, dtype=np.float32))
    bf16 = ml_dtypes.bfloat16
    h = f32(hidden_states)
    mask = f32(attention_mask)
    Wd1, bd1, Wd2, bd2 = f32(Wd1), f32(bd1), f32(Wd2), f32(bd2)
    We, be, Wi1, bi1, Wi2, bi2 = f32(We), f32(be), f32(Wi1), f32(bi1), f32(Wi2), f32(bi2)

    Bv, Sv, Hv = h.shape
    Ev = Wd2.shape[1]
    TOK = Bv * Sv
    T = TOK // N_CORES
    KT = Hv // P
    include_be = bool(np.any(be))

    nc = _get_nc(T, Hv, Ev, include_be)

    hf = h.reshape(TOK, Hv)
    mf = mask.reshape(TOK)

    # Wd2 is [H, E]: pack to [P, KT, E]
    wd2_p = np.ascontiguousarray(
        Wd2.reshape(KT, P, Ev).transpose(1, 0, 2).astype(bf16))
    bias_p = lambda b: np.ascontiguousarray(b.reshape(KT, P).T)
    weights = dict(
        wd1=_pack_w(Wd1, bf16), bd1=bias_p(bd1),
        wd2=wd2_p, bd2=bd2.reshape(1, Ev),
        we=np.ascontiguousarray(
            np.stack([_pack_w(We[e], bf16) for e in range(Ev)])),
        be=be,
        wi1=_pack_w(Wi1, bf16), bi1=bias_p(bi1),
        wi2=_pack_w(Wi2, bf16), bi2=bias_p(bi2),
    )
    in_maps = []
    for c in range(N_CORES):
        m = dict(weights)
        hc = hf[c * T:(c + 1) * T]                        # [T, H]
        # [TB, P, KT, 512]: per-partition-contiguous 512-token blocks
        hTc = hc.T.reshape(KT, P, T // 512, 512).transpose(2, 1, 0, 3)
        m["hT"] = np.ascontiguousarray(hTc.astype(bf16))
        m["mask"] = np.ascontiguousarray(mf[c * T:(c + 1) * T].reshape(1, T))
        in_maps.append(m)

    # The first execution of a freshly-loaded NEFF occasionally trips a
    # transient NRT_EXEC_UNIT_UNRECOVERABLE on the axon worker; a retry after a
    # short pause has always succeeded, so tolerate a couple of those.
    last_exc = None
    for attempt in range(3):
        try:
            res = run_bass_kernel_spmd(nc, in_maps,
                                       core_ids=list(range(N_CORES)),
                                       trace=_trace)
            break
        except Exception as e:  # noqa: BLE001 - jax.errors.JaxRuntimeError
            last_exc = e
            if "UNAVAILABLE" not in str(e) and "unrecoverable" not in str(e):
                raise
            import time as _time
            _time.sleep(5 * (attempt + 1))
    else:
        raise last_exc
    out = np.empty((TOK, Hv), dtype=np.float32)
    for c in range(N_CORES):
        oc = res.results[c]["out"]                        # [P, MT, T]
        out[c * T:(c + 1) * T] = oc.transpose(1, 0, 2).reshape(Hv, T).T
    out = out.reshape(Bv, Sv, Hv)
    if _trace:
        kernel._last_results = res
    return out


# revision 33
# speedup vs baseline: 1.0064x; 1.0005x over previous
"""Trainium2 Bass kernel for the EnhancedMathematicalReasoning MoE-routing module.

Computation (per token t, hidden dim H=2048, E=8 experts, dense routing):
    a1     = gelu(h @ Wd1 + bd1)
    logits = a1 @ Wd2 + bd2
    op_w   = softmax(logits)
    comb   = sum_e op_w[:, e] * (h @ We[e] + be[e])
    out    = (gelu(comb @ Wi1 + bi1) @ Wi2 + bi2) * mask

Sharding: data-parallel over the 8192 tokens -> 1024 tokens per NeuronCore,
weights replicated, no collectives.

Layout strategy (P=128):
  - The host pre-transposes h to [H, T] per core and packs every weight into
    the [P, NB, KT, 256] chunk layout the kernel DMAs, so the device does
    ZERO layout transposes: activations live as [H-on-partitions, T] from
    first GEMM to final store, and the output is written [H, T] and
    transposed back on the host.
  - ALL four GEMM stages run with bf16 operands at N=512 (fp32 PSUM
    accumulation).  bf16 matmuls pace at the 216 ns warm floor because the
    fast-weight-load path engages (fp32r paces at 227 ns/MM, LDW-limited),
    and bf16 halves all weight DMA.
  - Expert outputs are combined op_w-weighted into an fp32 arena (DVE
    mult+add per psum); the LAST expert's accumulate writes the bf16 copy
    (arenaB) that GEMM3 streams, so the 8-way accumulation itself stays
    fp32 with no extra cast pass.
  - op_w[t, e] is broadcast across partitions with a K=8 selector matmul
    (sel8), and the attention mask is broadcast across partitions once with
    a K=1 ones matmul; the GEMM4 eviction is ACT(bias) + DVE(mask mult).
  - Startup: one accumulation group of dummy matmuls warms the PE HAM
    clock-gate while hT and Wd1-chunk-0 transfer concurrently on the two
    hardware DMA queues (Sync + Activation); first real MM at ~12 us warm.
"""

import numpy as np
import ml_dtypes
from contextlib import ExitStack

import concourse.bass as bass
import concourse.tile as tile
from concourse import bacc, mybir
from concourse.bass_utils import run_bass_kernel_spmd
from concourse.masks import make_identity

F32 = mybir.dt.float32
F32R = mybir.dt.float32r
BF16 = mybir.dt.bfloat16
AF = mybir.ActivationFunctionType
ALU = mybir.AluOpType
AX = mybir.AxisListType

P = 128
N_CORES = 8

B, S, H_FULL, E_FULL = 4, 2048, 2048, 8


def build_nc(T, H, E, act=AF.Gelu, include_be=False):
    """Build + compile the single-core program (same program runs SPMD on all
    cores). T: tokens per core. Requires T % 512 == 0, H % 512 == 0."""
    assert T % 512 == 0 and H % 512 == 0 and 2 <= E <= P
    KT = H // P          # contraction chunks of 128
    TT = T // P          # token chunks of 128
    TB = T // 512        # 512-wide moving blocks
    NB = H // 256        # 256-wide weight chunks
    MT = H // P          # output chunks of 128

    nc = bacc.Bacc("TRN2", target_bir_lowering=False, debug=False)

    hT_d = nc.dram_tensor("hT", [T // 512, P, KT, 512], BF16,
                          kind="ExternalInput").ap()
    msk_d = nc.dram_tensor("mask", [1, T], F32R, kind="ExternalInput").ap()
    wd1_d = nc.dram_tensor("wd1", [P, NB, KT, 256], BF16, kind="ExternalInput").ap()
    bd1_d = nc.dram_tensor("bd1", [P, KT], F32, kind="ExternalInput").ap()
    wd2_d = nc.dram_tensor("wd2", [P, KT, E], BF16, kind="ExternalInput").ap()
    bd2_d = nc.dram_tensor("bd2", [1, E], F32, kind="ExternalInput").ap()
    we_d = nc.dram_tensor("we", [E, P, NB, KT, 256], BF16, kind="ExternalInput").ap()
    be_d = nc.dram_tensor("be", [E, H], F32R, kind="ExternalInput").ap()
    wi1_d = nc.dram_tensor("wi1", [P, NB, KT, 256], BF16, kind="ExternalInput").ap()
    bi1_d = nc.dram_tensor("bi1", [P, KT], F32, kind="ExternalInput").ap()
    wi2_d = nc.dram_tensor("wi2", [P, NB, KT, 256], BF16, kind="ExternalInput").ap()
    bi2_d = nc.dram_tensor("bi2", [P, KT], F32, kind="ExternalInput").ap()
    out_d = nc.dram_tensor("out", [P, MT, T], F32, kind="ExternalOutput").ap()

    with tile.TileContext(nc) as tc:
        with ExitStack() as ctx:
            const = ctx.enter_context(tc.tile_pool(name="const", bufs=1))
            bigp = ctx.enter_context(tc.tile_pool(name="bigp", bufs=1))
            web = ctx.enter_context(tc.tile_pool(name="web", bufs=3))
            a1p = ctx.enter_context(tc.tile_pool(name="a1p", bufs=3))
            scr = ctx.enter_context(tc.tile_pool(name="scr", bufs=3))
            opb = ctx.enter_context(tc.tile_pool(name="opb", bufs=3))
            smp = ctx.enter_context(tc.tile_pool(name="smp", bufs=4))
            pp = ctx.enter_context(tc.tile_pool(name="pp", bufs=2, space="PSUM"))

            # ---- constants (engine-generated; no DMA) ----
            ident = const.tile([P, P], F32, name="ident")
            make_identity(nc, ident)
            ones1 = const.tile([1, P], F32, name="ones1")
            nc.vector.memset(ones1, 1.0)
            # sel8[e', e*128+p] = (e' == e): K=8 selector used to broadcast
            # op_w rows across all 128 partitions via a tiny matmul.
            sel8f = const.tile([E, E, P], F32, name="sel8f")
            nc.gpsimd.memset(sel8f, 0.0)
            nc.gpsimd.affine_select(
                out=sel8f, in_=sel8f, compare_op=ALU.not_equal, fill=1.0,
                base=0, pattern=[[-1, E], [0, P]], channel_multiplier=1)
            sel8 = const.tile([E, E * P], F32R, name="sel8")
            nc.scalar.copy(sel8, sel8f.rearrange("e a p -> e (a p)"))
            opwT = const.tile([E, T], F32R, name="opwT")
            opw = const.tile([P, TT, E], F32, name="opw")
            lacc = const.tile([P, TT, E], F32, name="lacc")
            wmup5 = const.tile([P, 512], BF16, name="wmup5")
            nc.vector.memset(wmup5, 0.0)
            wmup = wmup5[:, 0:P]

            # ---- HAM warmup: one accumulation group of dummy matmuls (they
            # pipeline back-to-back with no WAW serialization) keeps the PE
            # activity monitor busy during the ~9us initial DMA so the first
            # real GEMM runs warm.  Borrows the logits psum bank.
            wps = pp.tile([P, 512], F32, tag="lgt", bufs=1, name="wps")
            for i in range(32):
                nc.tensor.matmul(wps, wmup, wmup5, start=(i == 0),
                                 stop=(i == 31))

            # ---- critical-path DMAs first, split across the two hardware
            # DMA queues (Sync and Activation) so hT and Wd1 chunk 0
            # transfer concurrently ----
            hT = bigp.tile([P, KT, T], BF16, tag="A", name="hT")
            w1_0 = web.tile([P, KT, 256], BF16, tag="web", name="wd1m_0")
            nc.sync.dma_start(hT[:, :, 0:512], hT_d[0])
            nc.scalar.dma_start(w1_0, wd1_d[:, 0])
            bd1_t = const.tile([P, KT], F32, name="bd1_t")
            nc.scalar.dma_start(bd1_t, bd1_d)
            wd2_t = const.tile([P, KT, E], BF16, name="wd2_t")
            nc.scalar.dma_start(wd2_t, wd2_d)
            bd2_t = const.tile([1, E], F32, name="bd2_t")
            nc.scalar.dma_start(bd2_t, bd2_d)
            for tb in range(1, TB):
                nc.sync.dma_start(hT[:, :, tb * 512:(tb + 1) * 512], hT_d[tb])

            # ---- mask broadcast across partitions: maskb[p, t] = mask[t] ----
            mask_sb = const.tile([1, T], F32R, name="mask_sb")
            nc.sync.dma_start(mask_sb, msk_d)
            maskb = const.tile([P, T], F32, name="maskb")
            ones1R = const.tile([1, P], F32R, name="ones1R")
            nc.scalar.copy(ones1R, ones1)
            for tb in range(TB):
                mbps = pp.tile([P, 512], F32, tag="tr", bufs=2, name="mbps")
                nc.tensor.matmul(mbps, ones1R,
                                 mask_sb[:, tb * 512:(tb + 1) * 512],
                                 start=True, stop=True)
                nc.scalar.copy(maskb[:, tb * 512:(tb + 1) * 512], mbps)

            # ---- remaining small constants ----
            bi1_t = const.tile([P, KT], F32, name="bi1_t")
            nc.sync.dma_start(bi1_t, bi1_d)
            bi2_t = const.tile([P, KT], F32, name="bi2_t")
            nc.sync.dma_start(bi2_t, bi2_d)
            if include_be:
                be_t = const.tile([E, H], F32R, name="be_t")
                nc.sync.dma_start(be_t, be_d)

            # ---- stage B: a1 = act(Wd1.T @ hT + bd1) + fused logits GEMM ----
            # tb-outer within each weight chunk so the first 32 MMs only need
            # hT's tb=0 half (the tb=1 DMA is still in flight at kernel start)
            for mg in range(NB):
                if mg == 0:
                    w1 = w1_0
                else:
                    w1 = web.tile([P, KT, 256], BF16, tag="web", name=f"wd1m_{mg}")
                    nc.sync.dma_start(w1, wd1_d[:, mg])
                a1s = {}
                for mi in range(2):
                    m = 2 * mg + mi
                    a1s[mi] = a1p.tile([P, T], BF16, tag="a1", bufs=3,
                                       name=f"a1_{m}")
                for tb in range(TB):
                    for mi in range(2):
                        m = 2 * mg + mi
                        ps = pp.tile([P, 512], F32, tag="mm", bufs=5, name="ps_g1")
                        for k in range(KT):
                            nc.tensor.matmul(ps, w1[:, k, mi * P:(mi + 1) * P],
                                             hT[:, k, tb * 512:(tb + 1) * 512],
                                             start=(k == 0), stop=(k == KT - 1))
                        nc.scalar.activation(a1s[mi][:, tb * 512:(tb + 1) * 512],
                                             ps, act, bias=bd1_t[:, m:m + 1])
                for mi in range(2):
                    m = 2 * mg + mi
                    a1 = a1s[mi]
                    for tt in range(TT):
                        lg = pp.tile([P, E], F32, tag="lgt", bufs=1, name="lg")
                        nc.tensor.matmul(lg, a1[:, tt * P:(tt + 1) * P],
                                         wd2_t[:, m, :], start=True, stop=(m > 0))
                        if m == 0:
                            nc.tensor.matmul(lg, ones1, bd2_t, start=False,
                                             stop=True)
                            nc.vector.tensor_copy(lacc[:, tt, :], lg)
                        else:
                            nc.vector.tensor_add(lacc[:, tt, :], lacc[:, tt, :], lg)

            # ---- softmax over E ----
            for tt in range(TT):
                nmax = smp.tile([P, 1], F32, tag="sm1", bufs=6, name="nmax")
                nc.vector.reduce_max(nmax, lacc[:, tt, :], AX.X, negate=True)
                et = smp.tile([P, E], F32, tag="sme", bufs=2, name="et")
                nc.scalar.activation(et, lacc[:, tt, :], AF.Exp, bias=nmax,
                                     scale=1.0)
                ssum = smp.tile([P, 1], F32, tag="sm1", bufs=6, name="ssum")
                nc.vector.reduce_sum(ssum, et, AX.X)
                rin = smp.tile([P, 1], F32, tag="sm1", bufs=6, name="rin")
                nc.vector.reciprocal(rin, ssum)
                nc.vector.tensor_scalar_mul(opw[:, tt, :], et, rin)
            # Hoisted: expert-0 chunk-0 psum fills (m=0,1 x tb) depend only on
            # hT and We[0], so they keep the PE busy while the serial softmax
            # -> op_w-transpose -> broadcast chain resolves on DVE/ACT.
            wet00 = web.tile([P, KT, 256], BF16, tag="web", name="we_0_0")
            nc.sync.dma_start(wet00, we_d[0, :, 0])
            hoist_ps = {}
            for mi in range(2):
                for tb in range(TB):
                    ps = pp.tile([P, 512], F32, tag="mm", bufs=5, name="eps_h")
                    for k in range(KT):
                        nc.tensor.matmul(ps, wet00[:, k, mi * P:(mi + 1) * P],
                                         hT[:, k, tb * 512:(tb + 1) * 512],
                                         start=(k == 0), stop=(k == KT - 1))
                    hoist_ps[(mi, tb)] = ps
            for tt in range(TT):
                trp = pp.tile([P, P], F32, tag="tr", bufs=2, name="trp_ow")
                nc.tensor.matmul(trp[:E, :], opw[:, tt, :], ident,
                                 is_transpose=True, start=True, stop=True)
                nc.scalar.copy(opwT[:, tt * P:(tt + 1) * P], trp[:E, :])

            # ---- stage C: expert GEMMs, [H_out, T] psums.
            # op_w[t, e] is broadcast across partitions as ob = sel8[:, e].T @
            # opwT (a K=8 matmul), and the weighted combine accumulates into
            # the fp32 arena combT [H, T]:
            #     arena[m, t] (+)= ob[t] * psum[m, t]
            arena = bigp.tile([P, KT, TT, P], F32, tag="B", name="arena")
            arenaR = arena.bitcast(F32R)
            # bf16 copy of the finished comb, written directly by the LAST
            # expert's accumulate (no separate cast pass); GEMM3's moving
            # operand, laid out like hT: [P, KT, T]
            arenaB = bigp.tile([P, KT, T], BF16, tag="C", name="arenaB")

            if include_be:
                # init combT with the op_w-weighted bias term:
                #   arena[m*128+p, t] = sum_e op_w[t, e] * be[e, m*128+p]
                for m in range(MT):
                    for tb in range(TB):
                        bps = pp.tile([P, 512], F32, tag="mm", bufs=5, name="bps")
                        nc.tensor.matmul(bps, be_t[:, m * P:(m + 1) * P],
                                         opwT[:, tb * 512:(tb + 1) * 512],
                                         start=True, stop=True)
                        nc.scalar.copy(
                            arenaR[:, m, tb * 4:(tb + 1) * 4, :],
                            bps.rearrange("p (n c) -> p n c", c=P))

            obs = {}
            for e in range(E):
                for tb in range(TB):
                    bps = pp.tile([P, 512], F32, tag="tr", bufs=2, name="bps")
                    nc.tensor.matmul(bps, sel8[:, e * P:(e + 1) * P],
                                     opwT[:, tb * 512:(tb + 1) * 512],
                                     start=True, stop=True)
                    ob = opb.tile([P, 512], F32, tag="ob", bufs=3,
                                  name=f"ob_{e}_{tb}")
                    nc.scalar.copy(ob, bps)
                    obs[tb] = ob
                for mg in range(NB):
                    if e == 0 and mg == 0:
                        wet = wet00
                    else:
                        wet = web.tile([P, KT, 256], BF16, tag="web",
                                       name=f"we_{e}_{mg}")
                        nc.sync.dma_start(wet, we_d[e, :, mg])
                    for mi in range(2):
                        m = 2 * mg + mi
                        for tb in range(TB):
                            if e == 0 and mg == 0:
                                ps = hoist_ps[(mi, tb)]
                            else:
                                ps = pp.tile([P, 512], F32, tag="mm", bufs=5,
                                             name="eps")
                                for k in range(KT):
                                    nc.tensor.matmul(
                                        ps, wet[:, k, mi * P:(mi + 1) * P],
                                        hT[:, k, tb * 512:(tb + 1) * 512],
                                        start=(k == 0), stop=(k == KT - 1))
                            wsl = arenaR[:, m, tb * 4:(tb + 1) * 4, :]
                            rsl = arena[:, m, tb * 4:(tb + 1) * 4, :]
                            ob3 = obs[tb].rearrange("p (n c) -> p n c", c=P)
                            ps3 = ps.rearrange("p (n c) -> p n c", c=P)
                            if e == 0 and not include_be:
                                nc.vector.tensor_tensor(wsl, ps3, ob3,
                                                        op=ALU.mult)
                            else:
                                tmp = scr.tile([P, 512], F32, tag="s",
                                               bufs=3, name="tmp")
                                tmp3 = tmp.rearrange("p (n c) -> p n c", c=P)
                                nc.vector.tensor_tensor(tmp3, ps3, ob3,
                                                        op=ALU.mult)
                                if e == E - 1:
                                    wb3 = arenaB[
                                        :, m, tb * 512:(tb + 1) * 512
                                    ].rearrange("p (n c) -> p n c", c=P)
                                    nc.vector.tensor_tensor(wb3, rsl, tmp3,
                                                            op=ALU.add)
                                else:
                                    nc.vector.tensor_tensor(wsl, rsl, tmp3,
                                                            op=ALU.add)
                    # prefetch the first Wi1 chunk near the end of the last
                    # expert so stage E doesn't stall on DMA
                    if e == E - 1 and mg == NB - 2:
                        w3_0 = web.tile([P, KT, 256], BF16, tag="web",
                                        name="wi1m_0")
                        nc.sync.dma_start(w3_0, wi1_d[:, 0])

            # ---- stage E: a2T = act(Wi1.T @ arenaB + bi1), bf16 ----
            a2T = bigp.tile([P, KT, T], BF16, tag="A", name="a2T")
            for mg in range(NB):
                if mg == 0:
                    w3 = w3_0
                else:
                    w3 = web.tile([P, KT, 256], BF16, tag="web",
                                  name=f"wi1m_{mg}")
                    nc.sync.dma_start(w3, wi1_d[:, mg])
                for mi in range(2):
                    m = 2 * mg + mi
                    for tb in range(TB):
                        ps = pp.tile([P, 512], F32, tag="mm", bufs=5, name="ps_g3")
                        for k in range(KT):
                            nc.tensor.matmul(ps, w3[:, k, mi * P:(mi + 1) * P],
                                             arenaB[:, k, tb * 512:(tb + 1) * 512],
                                             start=(k == 0), stop=(k == KT - 1))
                        nc.scalar.activation(a2T[:, m, tb * 512:(tb + 1) * 512],
                                             ps, act, bias=bi1_t[:, m:m + 1])
                # prefetch the first Wi2 chunk before stage F
                if mg == NB - 2:
                    w4_0 = web.tile([P, KT, 256], BF16, tag="web", name="wi2m_0")
                    nc.sync.dma_start(w4_0, wi2_d[:, 0])

            # ---- stage F: out = (Wi2.T @ a2T + bi2) * maskb, stored [H, T] ----
            for mg in range(NB):
                if mg == 0:
                    w4 = w4_0
                else:
                    w4 = web.tile([P, KT, 256], BF16, tag="web", name=f"wi2m_{mg}")
                    nc.sync.dma_start(w4, wi2_d[:, mg])
                for mi in range(2):
                    m = 2 * mg + mi
                    for tb in range(TB):
                        ps = pp.tile([P, 512], F32, tag="mm", bufs=5, name="ps_g4")
                        for k in range(KT):
                            nc.tensor.matmul(ps, w4[:, k, mi * P:(mi + 1) * P],
                                             a2T[:, k, tb * 512:(tb + 1) * 512],
                                             start=(k == 0), stop=(k == KT - 1))
                        og = scr.tile([P, 512], F32, tag="og", bufs=3, name="og")
                        nc.scalar.activation(og, ps, AF.Identity,
                                             bias=bi2_t[:, m:m + 1])
                        og2 = scr.tile([P, 512], F32, tag="og2", bufs=3,
                                       name="og2")
                        nc.vector.tensor_tensor(
                            og2, og, maskb[:, tb * 512:(tb + 1) * 512],
                            op=ALU.mult)
                        nc.sync.dma_start(
                            out_d[:, m, tb * 512:(tb + 1) * 512], og2)

    nc.compile()
    return nc


_CACHED = {}


def _get_nc(T, H, E, include_be):
    key = (T, H, E, include_be)
    if key not in _CACHED:
        _CACHED[key] = build_nc(T, H, E, act=AF.Gelu, include_be=include_be)
    return _CACHED[key]


def _pack_w(W, dtype):
    """[H_in, N_out] -> [P, NB, KT, 256] chunk layout (contiguous per DMA)."""
    H_in, N_out = W.shape
    KT, NB = H_in // P, N_out // 256
    Wp = W.reshape(KT, P, NB, 256).transpose(1, 2, 0, 3)
    return np.ascontiguousarray(Wp.astype(dtype))


def kernel(hidden_states, attention_mask, Wd1, bd1, Wd2, bd2, We, be, Wi1, bi1,
           Wi2, bi2, _trace=False):
    f32 = lambda x: np.ascontiguousarray(np.asarray(x, dtype=np.float32))
    bf16 = ml_dtypes.bfloat16
    h = f32(hidden_states)
    mask = f32(attention_mask)
    Wd1, bd1, Wd2, bd2 = f32(Wd1), f32(bd1), f32(Wd2), f32(bd2)
    We, be, Wi1, bi1, Wi2, bi2 = f32(We), f32(be), f32(Wi1), f32(bi1), f32(Wi2), f32(bi2)

    Bv, Sv, Hv = h.shape
    Ev = Wd2.shape[1]
    TOK = Bv * Sv
    T = TOK // N_CORES
    KT = Hv // P
    include_be = bool(np.any(be))

    nc = _get_nc(T, Hv, Ev, include_be)

    hf = h.reshape(TOK, Hv)
    mf = mask.reshape(TOK)

    # Wd2 is [H, E]: pack to [P, KT, E]
    wd2_p = np.ascontiguousarray(
        Wd2.reshape(KT, P, Ev).transpose(1, 0, 2).astype(bf16))
    bias_p = lambda b: np.ascontiguousarray(b.reshape(KT, P).T)
    weights = dict(
        wd1=_pack_w(Wd1, bf16), bd1=bias_p(bd1),
        wd2=wd2_p, bd2=bd2.reshape(1, Ev),
        we=np.ascontiguousarray(
            np.stack([_pack_w(We[e], bf16) for e in range(Ev)])),
        be=be,
        wi1=_pack_w(Wi1, bf16), bi1=bias_p(bi1),
        wi2=_pack_w(Wi2, bf16), bi2=bias_p(bi2),
    )
    in_maps = []
    for c in range(N_CORES):
        m = dict(weights)
        hc = hf[c * T:(c + 1) * T]                        # [T, H]
        # [TB, P, KT, 512]: per-partition-contiguous 512-token blocks
        hTc = hc.T.reshape(KT, P, T // 512, 512).transpose(2, 1, 0, 3)
        m["hT"] = np.ascontiguousarray(hTc.astype(bf16))
        m["mask"] = np.ascontiguousarray(mf[c * T:(c + 1) * T].reshape(1, T))
        in_maps.append(m)

    # The first execution of a freshly-loaded NEFF occasionally trips a
    # transient NRT_EXEC_UNIT_UNRECOVERABLE on the axon worker; a retry after a
    # short pause has always succeeded, so tolerate a couple of those.
    last_exc = None
    for attempt in range(3):
        try:
            res = run_bass_kernel_spmd(nc, in_maps,
                                       core_ids=list(range(N_CORES)),
                                       trace=_trace)
            break
        except Exception as e:  # noqa: BLE001 - jax.errors.JaxRuntimeError
            last_exc = e
            if "UNAVAILABLE" not in str(e) and "unrecoverable" not in str(e):
                raise
            import time as _time
            _time.sleep(5 * (attempt + 1))
    else:
        raise last_exc
    out = np.empty((TOK, Hv), dtype=np.float32)
    for c in range(N_CORES):
        oc = res.results[c]["out"]                        # [P, MT, T]
        out[c * T:(c + 1) * T] = oc.transpose(1, 0, 2).reshape(Hv, T).T
    out = out.reshape(Bv, Sv, Hv)
    if _trace:
        kernel._last_results = res
    return out
